# revision 28
# baseline (speedup 1.0000x reference)
"""Causal self-attention (B=2, T=2048, C=1024, H=16) on 8 TRN2 NeuronCores.

Sharding: data-parallel over batch (2 groups of 4 cores) x tensor-parallel
over heads (4 heads per core, Megatron-style column/row split of the
qkv / proj weights). Each core computes, for its (batch, head-group):

    qkT/kT  = (W_qk^T x^T + b_qk)        [512, 2048]   (transposed layout)
    v       = x W_v + b_v                [2048, 256]
    S^T     = kT^T-tiles x qT            per (head, i-chunk) blocks
    P       = exp(S^T / 8) * causal_mask (no max-subtraction: logits are
                                          small, |S/8| < ~3, exp is safe)
    Yu^T    = [v | 1]^T P^T              rows 0..63 unnormalized y^T,
                                          row 64 = softmax denominator
    y^T     = Yu^T * (1/denom)           (denominator broadcast across
                                          partitions via gpsimd)
    out^T  += W_p-rows^T y^T             [1024, 2048] partial projection

The host sums the 4 partial projections per batch and adds b_proj.

Schedule: one software-pipelined stream.  The attention j-tile stream of
chunk c is interleaved (in emission order, which seeds the Tile list
scheduler's priorities) with the qkv matmuls of chunk c+1 and the
projection of chunk c-1, so the PE never drains while the scalar engine
works through the exp stream (exp is 1 elem/lane/cycle @1.2GHz and is
the pacing engine inside attention phases).

The two heads of a pair sit in partition halves 0:64 / 64:128 of the
same qk tile, so their K=64 S-matmuls are emitted back-to-back and run
CONCURRENTLY in disjoint 64-row PE tile groups (tile_position is
auto-derived from the operand base partitions) -- S costs ~1x N cycles
per head pair instead of 2x.

Engine placement: PE matmuls; scalar ONLY exp; vector bias adds, copies,
reciprocal, normalize; gpsimd causal masking (affine_select on the
post-exp P, which writes zeros exactly where the mask would) and the
denominator partition-broadcast.  Initial weight/x DMAs are spread over
the sync/scalar/vector/gpsimd queues so the startup is not serialized on
one DMA ring.
"""

import numpy as np

B, T, C, H = 2, 2048, 1024, 16
HD = C // H  # 64
HG = 4  # head-groups (tensor-parallel degree); B * HG = 8 cores
HPG = H // HG  # heads per group = 4
DG = HPG * HD  # columns per group = 256
TC = 512  # t-chunk (moving free dim)
NTC = T // TC  # 4
NJT = T // 128  # 16 j-tiles of 128 keys

_cached = {}


def _build():
    import concourse.mybir as mybir
    import concourse.tile as tile
    from concourse import bacc

    F32 = mybir.dt.float32
    F32R = mybir.dt.float32r
    BF16 = mybir.dt.bfloat16
    Act = mybir.ActivationFunctionType

    nc = bacc.Bacc()
    x_d = nc.declare_dram_parameter("xt", [C, T], BF16, isOutput=False)
    wqk_d = nc.declare_dram_parameter("wqk", [C, 2 * DG], BF16, isOutput=False)
    bqk_d = nc.declare_dram_parameter("bqk", [2 * DG, 1], F32, isOutput=False)
    wv_d = nc.declare_dram_parameter("wv", [C, DG], BF16, isOutput=False)
    bv_d = nc.declare_dram_parameter("bv", [1, DG], F32, isOutput=False)
    wp_d = nc.declare_dram_parameter("wp", [DG, C], BF16, isOutput=False)
    out_d = nc.declare_dram_parameter("outt", [C, T], BF16, isOutput=True)

    with tile.TileContext(nc) as tc:
        with (
            tc.tile_pool(name="const", bufs=1) as const,
            tc.tile_pool(name="sb", bufs=1) as sb,
            tc.tile_pool(name="ps", bufs=1, space="PSUM") as ps,
        ):
            # ---- weights ----
            wqk_sb = const.tile([128, 8, 2 * DG], BF16)
            wv_sb = const.tile([128, 8, DG], BF16)
            wp_sb = const.tile([128, 2, C], BF16)
            bqk_sb = const.tile([128, 4], F32)
            bv_sb = const.tile([128, HPG, HD], F32)

            # biases first (tiny, they gate the first bias-adds)
            nc.sync.dma_start(
                out=bqk_sb,
                in_=bqk_d[:, :].rearrange("(t p) o -> p (t o)", p=128),
            )
            for h in range(HPG):
                nc.gpsimd.dma_start(
                    out=bv_sb[:, h, :],
                    in_=bv_d[0:1, h * HD : (h + 1) * HD].to_broadcast((128, HD)),
                )

            # ---- persistent activations ----
            # qkT [j, t]: j-tiles 0,1 = q (head pairs 0,1), 2,3 = k
            qk_sb = const.tile([128, 4, T], BF16)
            # v1 [t, d]: per (t-tile, head): 64 v columns + ones column
            v1_sb = const.tile([128, NJT, HPG, HD + 1], BF16)
            # y^T [c', t]: c' = head-major 256 rows in 2 tiles
            yt_sb = const.tile([128, 2, T], BF16)
            nc.vector.memset(v1_sb[:, :, :, HD : HD + 1], 1.0)

            # x arrives pre-transposed from the host: [C, T], c on partitions
            xt_tiles = {}
            masks = []
            ones_bc = const.tile([1, HD], F32)
            nc.vector.memset(ones_bc, 1.0)

            def build_xt(tcx):
                xt = sb.tile(
                    [128, 8, TC], BF16, tag="xt", bufs=2, name=f"xt{tcx}"
                )
                # per-kc [128, 512] transfers keep the DRAM side contiguous
                # (a rearranged batch interleaves rows -> ~2-3x slower on the
                # wire); during startup split across two queues so the first
                # qkv matmuls are fed ASAP, later the scalar queue must stay
                # clear for the exp stream
                for kc in range(8):
                    eng = nc.sync if (tcx > 1 or kc % 2 == 0) else nc.scalar
                    eng.dma_start(
                        out=xt[:, kc, :],
                        in_=x_d[
                            kc * 128 : (kc + 1) * 128,
                            tcx * TC : (tcx + 1) * TC,
                        ],
                    )
                xt_tiles[tcx] = xt

            # initial loads interleaved in first-needed order over three
            # parallel queues
            xt0 = sb.tile([128, 8, TC], BF16, tag="xt", bufs=2, name="xt0")
            xt_tiles[0] = xt0
            engs = [nc.sync, nc.scalar, nc.gpsimd]
            for kc in range(8):
                engs[(2 * kc) % 3].dma_start(
                    out=xt0[:, kc, :],
                    in_=x_d[kc * 128 : (kc + 1) * 128, 0:TC],
                )
                engs[(2 * kc + 1) % 3].dma_start(
                    out=wqk_sb[:, kc, :], in_=wqk_d[kc * 128 : (kc + 1) * 128, :]
                )
            for kc in range(8):
                engs[kc % 3].dma_start(
                    out=wv_sb[:, kc, :], in_=wv_d[kc * 128 : (kc + 1) * 128, :]
                )
            nc.gpsimd.dma_start(
                out=wp_sb[:, 0, :], in_=wp_d[0:128, :]
            )
            nc.sync.dma_start(
                out=wp_sb[:, 1, :], in_=wp_d[128:256, :]
            )

            # ---- filler groups (qkv / proj work woven into attention) ----
            def qk_group(tcx, jt):
                def run():
                    xt = xt_tiles[tcx]
                    pqk = ps.tile([128, TC], F32, tag="mm", bufs=2, uniquify=True)
                    for kc in range(8):
                        nc.tensor.matmul(
                            pqk,
                            wqk_sb[:, kc, jt * 128 : (jt + 1) * 128],
                            xt[:, kc, :],
                            start=(kc == 0),
                            stop=(kc == 7),
                        )
                    nc.vector.tensor_scalar_add(
                        qk_sb[:, jt, tcx * TC : (tcx + 1) * TC],
                        pqk,
                        bqk_sb[:, jt : jt + 1],
                    )
                return run

            def v_group(tcx, tt):
                def run():
                    xt = xt_tiles[tcx]
                    pv = ps.tile(
                        [128, HPG, HD], F32, tag="mm", bufs=2, uniquify=True
                    )
                    for kc in range(8):
                        nc.tensor.matmul(
                            pv,
                            xt[:, kc, tt * 128 : (tt + 1) * 128],
                            wv_sb[:, kc, :],
                            start=(kc == 0),
                            stop=(kc == 7),
                        )
                    nc.vector.tensor_add(
                        v1_sb[:, tcx * 4 + tt, :, 0:HD], pv, bv_sb
                    )
                return run

            def proj_group(tcx, mt):
                def run():
                    po = ps.tile([128, TC], F32, tag="mm", bufs=2, uniquify=True)
                    for cc in range(2):
                        nc.tensor.matmul(
                            po,
                            wp_sb[:, cc, mt * 128 : (mt + 1) * 128],
                            yt_sb[:, cc, tcx * TC : (tcx + 1) * TC],
                            start=(cc == 0),
                            stop=(cc == 1),
                        )
                    ot = sb.tile([128, TC], BF16, tag="ot", bufs=3, uniquify=True)
                    nc.any.tensor_copy(ot, po)
                    (nc.gpsimd if mt % 2 == 0 else nc.sync).dma_start(
                        out=out_d[
                            mt * 128 : (mt + 1) * 128, tcx * TC : (tcx + 1) * TC
                        ],
                        in_=ot,
                    )
                return run

            def qkv_groups(tcx):
                # pair-0's q/k tiles first so pair-0 diagonals unblock early
                gs = [qk_group(tcx, 0), qk_group(tcx, 2)]
                gs += [v_group(tcx, tt) for tt in range(4)]
                gs += [qk_group(tcx, 1), qk_group(tcx, 3)]
                return gs

            # ---- attention ----
            # j-tiles are processed two at a time in a [128, 2*TC] "super
            # tile" spanning two PSUM banks, so ONE exp instruction covers
            # both -- the ACT fixed overhead (~352 cyc) is the dominant
            # scalar cost otherwise
            def attention_chunk(tcx, fillers):
                njt = 4 * (tcx + 1)
                nsup = njt // 2
                slots = 2 * nsup
                F = len(fillers)
                s = 0

                def drip():
                    nonlocal s
                    a = F * s // slots
                    b = F * (s + 1) // slots
                    for g in fillers[a:b]:
                        g()
                    s += 1

                def cut_of(jt):
                    kk = jt - 4 * tcx
                    return 0 if kk <= 0 else 128 * kk

                def emit_y_super(ent, p, pyA, pyB):
                    u, pA, pB = ent
                    for jl in range(2):
                        jt = 2 * u + jl
                        cut = cut_of(jt)
                        lo = jl * TC + cut
                        hi = (jl + 1) * TC
                        nc.tensor.matmul(
                            pyA[:, cut:],
                            v1_sb[:, jt, 2 * p, :],
                            pA[:, lo:hi],
                            start=(jt == 0),
                            stop=(jt == njt - 1),
                        )
                        nc.tensor.matmul(
                            pyB[:, cut:],
                            v1_sb[:, jt, 2 * p + 1, :],
                            pB[:, lo:hi],
                            start=(jt == 0),
                            stop=(jt == njt - 1),
                        )

                for p in range(2):
                    pyA = ps.tile(
                        [HD + 1, TC], F32, tag="py", bufs=2, uniquify=True
                    )
                    pyB = ps.tile(
                        [HD + 1, TC], F32, tag="py", bufs=2, uniquify=True
                    )
                    pend = []
                    for u in range(nsup):
                        cut0 = cut_of(2 * u)
                        # two heads of the pair: disjoint 64-row PE tile
                        # groups, concurrent on hardware; each [128, 2*TC]
                        # super tile holds j-tiles 2u and 2u+1
                        spA = ps.tile(
                            [128, 2 * TC], F32, tag="sp", bufs=2, uniquify=True
                        )
                        spB = ps.tile(
                            [128, 2 * TC], F32, tag="sp", bufs=2, uniquify=True
                        )
                        for jl in range(2):
                            jt = 2 * u + jl
                            cut = cut_of(jt)
                            lo = jl * TC + cut
                            hi = (jl + 1) * TC
                            for rows, sp in ((slice(0, 64), spA), (slice(64, 128), spB)):
                                nc.tensor.matmul(
                                    sp[:, lo:hi],
                                    qk_sb[rows, 2 + p, jt * 128 : (jt + 1) * 128],
                                    qk_sb[rows, p, tcx * TC + cut : (tcx + 1) * TC],
                                    start=True,
                                    stop=True,
                                )
                        pA = sb.tile(
                            [128, 2 * TC], BF16, tag="p", bufs=4, uniquify=True
                        )
                        pB = sb.tile(
                            [128, 2 * TC], BF16, tag="p", bufs=4, uniquify=True
                        )
                        # one exp per (head, super tile).  Any never-written
                        # PSUM columns inside [cut0:] produce garbage exp
                        # values, but the mask-mul and Y regions below start
                        # at each sub-tile's own cut, so garbage is never
                        # read.  Exception: the second diagonal super would
                        # stream 384 garbage columns through the ACT engine,
                        # more than the ~293ns instruction overhead saved --
                        # split it per sub-region instead.
                        if cut0 == 256:
                            for jl in range(2):
                                cut = cut_of(2 * u + jl)
                                lo = jl * TC + cut
                                hi = (jl + 1) * TC
                                nc.scalar.activation(
                                    pA[:, lo:hi], spA[:, lo:hi], Act.Exp, scale=0.125
                                )
                                nc.scalar.activation(
                                    pB[:, lo:hi], spB[:, lo:hi], Act.Exp, scale=0.125
                                )
                        else:
                            nc.scalar.activation(
                                pA[:, cut0:], spA[:, cut0:], Act.Exp, scale=0.125
                            )
                            nc.scalar.activation(
                                pB[:, cut0:], spB[:, cut0:], Act.Exp, scale=0.125
                            )
                        for jl in range(2):
                            jt = 2 * u + jl
                            kk = jt - 4 * tcx
                            if kk >= 0:
                                cut = cut_of(jt)
                                lo = jl * TC + cut
                                hi = (jl + 1) * TC
                                for pt in (pA, pB):
                                    nc.vector.tensor_mul(
                                        pt[:, lo:hi],
                                        pt[:, lo:hi],
                                        masks[kk][:, cut:],
                                    )
                        pend.append((u, pA, pB))
                        if len(pend) > 1:
                            emit_y_super(pend.pop(0), p, pyA, pyB)
                        drip()
                    for ent in pend:
                        emit_y_super(ent, p, pyA, pyB)
                    # normalize: the custom-DVE reciprocal cannot read PSUM
                    # on hardware -- stage the denominator row through SBUF
                    for row, py in ((0, pyA), (64, pyB)):
                        dn = sb.tile([1, TC], F32, tag="dn", bufs=4, uniquify=True)
                        nc.vector.tensor_copy(dn, py[HD : HD + 1, :])
                        rc = sb.tile([1, TC], F32, tag="rc", bufs=4, uniquify=True)
                        nc.vector.reciprocal_approx_fast(rc, dn)
                        rb = sb.tile(
                            [HD, TC], F32, tag="rb", bufs=4, uniquify=True
                        )
                        nc.gpsimd.partition_broadcast(rb, rc)
                        nc.vector.tensor_mul(
                            yt_sb[row : row + 64, p, tcx * TC : (tcx + 1) * TC],
                            py[0:HD, :],
                            rb,
                        )

            # ---- the pipelined stream ----
            for g in qkv_groups(0):
                g()

            # causal masks for the 4 diagonal sub-blocks of an i-chunk:
            # mask_k[j, i] = 1 iff i - 128*k - j >= 0 (built behind qkv(0),
            # needed first by attention(0))
            mask_f = const.tile([128, TC], F32)
            for k in range(4):
                nc.vector.memset(mask_f, 1.0)
                nc.gpsimd.affine_select(
                    out=mask_f,
                    in_=mask_f,
                    compare_op=mybir.AluOpType.is_ge,
                    fill=0.0,
                    base=-128 * k,
                    pattern=[[1, TC]],
                    channel_multiplier=-1,
                )
                mk = const.tile([128, TC], BF16, name=f"mask{k}", uniquify=True)
                nc.vector.tensor_copy(mk, mask_f)
                masks.append(mk)

            build_xt(1)
            attention_chunk(0, qkv_groups(1))
            build_xt(2)
            attention_chunk(1, [proj_group(0, mt) for mt in range(8)] + qkv_groups(2))
            build_xt(3)
            attention_chunk(2, [proj_group(1, mt) for mt in range(8)] + qkv_groups(3))
            attention_chunk(3, [proj_group(2, mt) for mt in range(8)])
            for mt in range(8):
                proj_group(3, mt)()

    nc.finalize()
    return nc


def _in_maps(x, W_attn, b_attn, W_proj):
    import ml_dtypes

    bf16 = ml_dtypes.bfloat16
    in_maps = []
    for core in range(8):
        b = core // HG
        hg = core % HG
        qs, ks, vs = hg * DG, C + hg * DG, 2 * C + hg * DG
        wqk = np.concatenate(
            [W_attn[:, qs : qs + DG], W_attn[:, ks : ks + DG]], axis=1
        )
        bqk = np.concatenate(
            [b_attn[qs : qs + DG], b_attn[ks : ks + DG]]
        ).reshape(2 * DG, 1)
        in_maps.append(
            {
                "xt": np.ascontiguousarray(x[b].T).astype(bf16),
                "wqk": np.ascontiguousarray(wqk).astype(bf16),
                "bqk": np.ascontiguousarray(bqk),
                "wv": np.ascontiguousarray(W_attn[:, vs : vs + DG]).astype(bf16),
                "bv": np.ascontiguousarray(b_attn[vs : vs + DG].reshape(1, DG)),
                "wp": np.ascontiguousarray(
                    W_proj[hg * DG : (hg + 1) * DG, :]
                ).astype(bf16),
            }
        )
    return in_maps


def _combine(results, b_proj):
    out = np.empty((B, T, C), dtype=np.float32)
    for b in range(B):
        acc = results[4 * b]["outt"].astype(np.float32)
        for hg in range(1, HG):
            acc = acc + results[4 * b + hg]["outt"].astype(np.float32)
        out[b] = acc.T + b_proj
    return out


def get_nc():
    if "nc" not in _cached:
        _cached["nc"] = _build()
    return _cached["nc"]


def kernel(x, W_attn, b_attn, W_proj, b_proj):
    from concourse.bass_utils import run_bass_kernel_spmd

    nc = get_nc()
    x = np.asarray(x, dtype=np.float32)
    W_attn = np.asarray(W_attn, dtype=np.float32)
    b_attn = np.asarray(b_attn, dtype=np.float32)
    W_proj = np.asarray(W_proj, dtype=np.float32)
    b_proj = np.asarray(b_proj, dtype=np.float32)

    in_maps = _in_maps(x, W_attn, b_attn, W_proj)
    r = run_bass_kernel_spmd(nc, in_maps, core_ids=list(range(8)), trace=False)
    return _combine(r.results, b_proj)
